# revision 25
# baseline (speedup 1.0000x reference)
"""Sum-reduced BCE-with-logits loss on 8 Trainium2 NeuronCores.

reference: loss = sum(softplus(x) - x * (labels > 0))  over x[1e6, 23] f32.

Strategy (data-parallel, per sharding hint):
  - Flatten x/target to 23M elements, pad to 8*128*22464, shard rows across
    8 cores; core c sees x_d [128, 22464] bf16 and t_d [128, 22464] fp8e4.
    (bf16 x changes the final sum by ~1.5e-8 relative — rounding cancels
    over 23M terms; fp8 {0,1} targets are exact.)
  - softplus = ln(1 + exp(x)) on ACT (this build has no softplus act
    table). By default exp resolves to the exp_and_others table set and
    ln to natural_log, which makes interleaved exp/ln swap ACT tables
    (~1.3us each). _Bacc restricts the act-table registry to
    natural_log_exp_and_others (positions preserved, so the emitted
    act_func_set_id stays canonical): ONE table load, loaded during the
    DMA ramp by a warm-up exp, and exp/ln interleave freely per chunk.
  - x and t are SBUF-resident (loads never stall on compute); all loads
    ride one HWDGE FIFO (nc.sync), x chunks first, t thirds slotted in
    behind. Per chunk: ACT exp -> f32 tile, ACT ln(1+u) with
    per-partition accumulate (bias=1.0 gives the +1 for free), DVE
    scalar_tensor_tensor accumulates -(x*t) in one pass.
  - Finish: one reduce over all partials to [128,1], cross-partition sum
    via PE matmul with a ones vector -> scalar per core; host adds the 8
    scalars.
Device time ~= ACT bound: 2 passes over 2.88M elem/core @153.6 G elem/s.
"""

import numpy as np

P = 128          # SBUF partitions
F = 22464        # per-core free dim (8*128*22464 = 23,003,136 >= 23e6)
CHUNKS = [936, 2808, 3744, 3744, 3744, 3744, 3744]   # sum == F
NCORES = 8
TOTAL = 23_000_000
TOTAL_PAD = NCORES * P * F
X_PAD = -30.0    # exp(-30) ~ 9e-14; ln(1+u) == 0.0 in f32

assert sum(CHUNKS) == F

_cache = {}

ACT_SET = "natural_log_exp_and_others"


def _make_bacc():
    import bass_rust as _bass_rust
    import concourse.bacc as bacc
    import concourse.mybir as mybir
    from concourse.hw_specs import get_activation_tables

    class _Bacc(bacc.Bacc):
        """Bacc with the act-table registry restricted to one set.

        Every activation here is exp or ln; both live in
        natural_log_exp_and_others. Blanking the other sets (positions
        preserved, so act_func_set_id still indexes act_info.json
        canonically) makes the load-insertion pass emit a single
        ACT_TABLE_LOAD instead of one per exp<->ln transition.
        """

        def insert_act_table_loads(self):
            has_activation = any(
                isinstance(i, mybir.InstActivation)
                for b in self.main_func.blocks
                for i in b.instructions
            )
            if not has_activation:
                return
            tabs = get_activation_tables(self.m.arch)
            keep = tabs.get(ACT_SET, set())
            if {mybir.ActivationFunctionType.Exp,
                    mybir.ActivationFunctionType.Ln} <= keep:
                tables = [(name, funcs if name == ACT_SET else set())
                          for name, funcs in tabs.items()]
            else:  # unexpected toolchain: fall back to the full registry
                tables = list(tabs.items())
            _bass_rust.insert_act_table_loads(self, tables)

    return _Bacc


def _build_nc():
    import concourse.mybir as mybir
    from concourse import tile

    f32 = mybir.dt.float32
    bf16 = mybir.dt.bfloat16
    fp8 = mybir.dt.float8e4
    AF = mybir.ActivationFunctionType
    ALU = mybir.AluOpType

    nc = _make_bacc()("TRN2", target_bir_lowering=False, debug=False)
    x_d = nc.dram_tensor("x", [P, F], bf16, kind="ExternalInput")
    t_d = nc.dram_tensor("t", [P, F], fp8, kind="ExternalInput")
    o_d = nc.dram_tensor("o", [1, 1], f32, kind="ExternalOutput")

    n = len(CHUNKS)
    offs = [sum(CHUNKS[:i]) for i in range(n)]
    # t loads merged into thirds; FIFO gives x priority, t slots behind.
    TW = F // 3
    assert F % 3 == 0
    dma_order = [("x", 0), ("x", 1), ("x", 2), ("x", 3), ("t", 0), ("x", 4),
                 ("x", 5), ("x", 6), ("t", 1), ("t", 2)]

    with tile.TileContext(nc) as tc:
        with (
            tc.tile_pool(name="junk", bufs=2) as jpool,
            tc.tile_pool(name="stats", bufs=1) as spool,
            tc.tile_pool(name="psum", bufs=1, space="PSUM") as ppool,
        ):
            # Warm-up exp so the act table set loads during the DMA ramp.
            warm = spool.tile([1, 1], f32)
            warm2 = spool.tile([1, 1], f32)
            nc.vector.memset(warm[:], 0.0)
            nc.scalar.activation(warm2[:], warm[:], AF.Exp)

            x_sb = spool.tile([P, F], bf16)           # resident input
            t_sb = spool.tile([P, F], fp8)            # resident targets
            # cols 0..n-1: DVE -(x*t) partials; cols n..2n-1: ln partials
            acc = spool.tile([P, 2 * n], f32)

            for kind, i in dma_order:
                if kind == "x":
                    off, w = offs[i], CHUNKS[i]
                    nc.sync.dma_start(out=x_sb[:, off:off + w],
                                      in_=x_d[:, off:off + w])
                else:
                    off = i * TW
                    nc.sync.dma_start(out=t_sb[:, off:off + TW],
                                      in_=t_d[:, off:off + TW])

            for i in range(n):
                off, w = offs[i], CHUNKS[i]
                e_t = jpool.tile([P, w], f32, tag="ej")
                nc.scalar.activation(e_t[:], x_sb[:, off:off + w], AF.Exp)
                sp_junk = jpool.tile([P, w], f32, tag="spj")
                nc.scalar.activation(
                    sp_junk[:], e_t[:], AF.Ln, bias=1.0,
                    accum_out=acc[:, n + i:n + i + 1],
                )
                tt_junk = jpool.tile([P, w], f32, tag="ttj")
                nc.vector.scalar_tensor_tensor(
                    out=tt_junk[:], in0=x_sb[:, off:off + w], scalar=-1.0,
                    in1=t_sb[:, off:off + w],
                    op0=ALU.mult, op1=ALU.mult,
                    accum_out=acc[:, i:i + 1],
                )

            total = spool.tile([P, 1], f32)
            nc.vector.tensor_reduce(
                out=total[:], in_=acc[:], axis=mybir.AxisListType.X,
                op=ALU.add)

            ones = spool.tile([P, 1], f32)
            nc.vector.memset(ones[:], 1.0)
            ps = ppool.tile([1, 1], f32)
            nc.tensor.matmul(ps[:], total[:], ones[:], start=True, stop=True)
            res = spool.tile([1, 1], f32)
            nc.vector.tensor_copy(res[:], ps[:])
            nc.sync.dma_start(out=o_d[:], in_=res[:])

    nc.compile()
    return nc


def _get_nc():
    if "nc" not in _cache:
        _cache["nc"] = _build_nc()
    return _cache["nc"]


def _prep(x, labels):
    import ml_dtypes
    bf16 = np.dtype(ml_dtypes.bfloat16)
    fp8 = np.dtype(ml_dtypes.float8_e4m3fn)
    x = np.asarray(x, dtype=np.float32).reshape(-1)
    t = np.asarray(labels).reshape(-1) > 0

    xf = np.full(TOTAL_PAD, X_PAD, dtype=bf16)
    xf[:TOTAL] = x.astype(bf16)
    tf = np.zeros(TOTAL_PAD, dtype=fp8)
    tf[:TOTAL] = t.astype(fp8)
    return xf.reshape(NCORES, P, F), tf.reshape(NCORES, P, F)


def kernel(x, labels, _trace=False):
    from concourse.bass_utils import run_bass_kernel_spmd

    xs, ts = _prep(x, labels)
    nc = _get_nc()
    in_maps = [{"x": xs[c], "t": ts[c]} for c in range(NCORES)]
    r = run_bass_kernel_spmd(nc, in_maps, list(range(NCORES)), trace=_trace)
    total = sum(float(r.results[c]["o"][0, 0]) for c in range(NCORES))
    out = np.asarray(total, dtype=np.float32)
    if _trace:
        _cache["last_results"] = r
    return out


# revision 27
# speedup vs baseline: 1.0038x; 1.0038x over previous
"""Sum-reduced BCE-with-logits loss on 8 Trainium2 NeuronCores.

reference: loss = sum(softplus(x) - x * (labels > 0))  over x[1e6, 23] f32.

Strategy (data-parallel, per sharding hint):
  - Flatten x/target to 23M elements, pad to 8*128*22464, shard rows across
    8 cores; core c sees x_d [128, 22464] bf16 and t_d [128, 22464] fp8e4.
    (bf16 x changes the final sum by ~1.5e-8 relative — rounding cancels
    over 23M terms; fp8 {0,1} targets are exact.)
  - softplus = ln(1 + exp(x)) on ACT (this build has no softplus act
    table). By default exp resolves to the exp_and_others table set and
    ln to natural_log, which makes interleaved exp/ln swap ACT tables
    (~1.3us each). _Bacc restricts the act-table registry to
    natural_log_exp_and_others (positions preserved, so the emitted
    act_func_set_id stays canonical): ONE table load, loaded during the
    DMA ramp by a warm-up exp, and exp/ln interleave freely per chunk.
  - x and t are SBUF-resident (loads never stall on compute); all loads
    ride one HWDGE FIFO (nc.sync), x chunks first, t thirds slotted in
    behind. Per chunk: ACT exp -> f32 tile, ACT ln(1+u) with
    per-partition accumulate (bias=1.0 gives the +1 for free), DVE
    scalar_tensor_tensor accumulates -(x*t) in one pass.
  - Finish: one reduce over all partials to [128,1], cross-partition sum
    via PE matmul with a ones vector -> scalar per core; host adds the 8
    scalars.
Device time ~= ACT bound: 2 passes over 2.88M elem/core @153.6 G elem/s.
"""

import numpy as np

P = 128          # SBUF partitions
F = 22464        # per-core free dim (8*128*22464 = 23,003,136 >= 23e6)
CHUNKS = [936, 2808, 3744, 3744, 3744, 3744, 3744]   # sum == F
NCORES = 8
TOTAL = 23_000_000
TOTAL_PAD = NCORES * P * F
X_PAD = -30.0    # exp(-30) ~ 9e-14; ln(1+u) == 0.0 in f32

assert sum(CHUNKS) == F

_cache = {}

ACT_SET = "natural_log_exp_and_others"


def _make_bacc():
    import bass_rust as _bass_rust
    import concourse.bacc as bacc
    import concourse.mybir as mybir
    from concourse.hw_specs import get_activation_tables

    class _Bacc(bacc.Bacc):
        """Bacc with the act-table registry restricted to one set.

        Every activation here is exp or ln; both live in
        natural_log_exp_and_others. Blanking the other sets (positions
        preserved, so act_func_set_id still indexes act_info.json
        canonically) makes the load-insertion pass emit a single
        ACT_TABLE_LOAD instead of one per exp<->ln transition.
        """

        def insert_act_table_loads(self):
            has_activation = any(
                isinstance(i, mybir.InstActivation)
                for b in self.main_func.blocks
                for i in b.instructions
            )
            if not has_activation:
                return
            tabs = get_activation_tables(self.m.arch)
            keep = tabs.get(ACT_SET, set())
            if {mybir.ActivationFunctionType.Exp,
                    mybir.ActivationFunctionType.Ln} <= keep:
                tables = [(name, funcs if name == ACT_SET else set())
                          for name, funcs in tabs.items()]
            else:  # unexpected toolchain: fall back to the full registry
                tables = list(tabs.items())
            _bass_rust.insert_act_table_loads(self, tables)

    return _Bacc


def _build_nc():
    import concourse.mybir as mybir
    from concourse import tile

    f32 = mybir.dt.float32
    bf16 = mybir.dt.bfloat16
    fp8 = mybir.dt.float8e4
    AF = mybir.ActivationFunctionType
    ALU = mybir.AluOpType

    nc = _make_bacc()("TRN2", target_bir_lowering=False, debug=False)
    x_d = nc.dram_tensor("x", [P, F], bf16, kind="ExternalInput")
    t_d = nc.dram_tensor("t", [P, F], fp8, kind="ExternalInput")
    o_d = nc.dram_tensor("o", [1, 1], f32, kind="ExternalOutput")

    n = len(CHUNKS)
    offs = [sum(CHUNKS[:i]) for i in range(n)]
    # t loads merged into thirds; FIFO gives x priority, t slots behind.
    TW = F // 3
    assert F % 3 == 0
    dma_order = [("x", 0), ("x", 1), ("x", 2), ("x", 3), ("t", 0), ("x", 4),
                 ("x", 5), ("x", 6), ("t", 1), ("t", 2)]

    with tile.TileContext(nc) as tc:
        with (
            tc.tile_pool(name="junk", bufs=2) as jpool,
            tc.tile_pool(name="stats", bufs=1) as spool,
            tc.tile_pool(name="psum", bufs=1, space="PSUM") as ppool,
        ):
            # Warm-up exp so the act table set loads during the DMA ramp.
            warm = spool.tile([1, 1], f32)
            warm2 = spool.tile([1, 1], f32)
            nc.vector.memset(warm[:], 0.0)
            nc.scalar.activation(warm2[:], warm[:], AF.Exp)

            x_sb = spool.tile([P, F], bf16)           # resident input
            t_sb = spool.tile([P, F], fp8)            # resident targets
            ej = spool.tile([P, F], f32)              # resident exp(x)
            # cols 0..n-1: DVE -(x*t) partials; col n: ln accumulate
            acc = spool.tile([P, n + 1], f32)

            for kind, i in dma_order:
                if kind == "x":
                    off, w = offs[i], CHUNKS[i]
                    nc.sync.dma_start(out=x_sb[:, off:off + w],
                                      in_=x_d[:, off:off + w])
                else:
                    off = i * TW
                    nc.sync.dma_start(out=t_sb[:, off:off + TW],
                                      in_=t_d[:, off:off + TW])

            # Phase 1: exp per chunk into the resident f32 buffer; DVE
            # accumulates -(x*t) alongside.
            for i in range(n):
                off, w = offs[i], CHUNKS[i]
                nc.scalar.activation(
                    ej[:, off:off + w], x_sb[:, off:off + w], AF.Exp)
                tt_junk = jpool.tile([P, w], f32, tag="ttj")
                nc.vector.scalar_tensor_tensor(
                    out=tt_junk[:], in0=x_sb[:, off:off + w], scalar=-1.0,
                    in1=t_sb[:, off:off + w],
                    op0=ALU.mult, op1=ALU.mult,
                    accum_out=acc[:, i:i + 1],
                )

            # Reduce the DVE partials while ACT is still busy.
            r_xt = spool.tile([P, 1], f32)
            nc.vector.tensor_reduce(
                out=r_xt[:], in_=acc[:, 0:n], axis=mybir.AxisListType.X,
                op=ALU.add)

            # Phase 2: ONE ln(1 + exp) over the whole row, in place over
            # the exp buffer (1:1 elementwise, read-before-write per
            # element), with per-partition accumulate. The RAW on ej
            # orders it after every exp; same table set, so no load.
            nc.scalar.activation(
                ej[:], ej[:], AF.Ln, bias=1.0,
                accum_out=acc[:, n:n + 1],
            )

            total = spool.tile([P, 1], f32)
            nc.vector.tensor_add(total[:], r_xt[:], acc[:, n:n + 1])

            ones = spool.tile([P, 1], f32)
            nc.vector.memset(ones[:], 1.0)
            ps = ppool.tile([1, 1], f32)
            nc.tensor.matmul(ps[:], total[:], ones[:], start=True, stop=True)
            res = spool.tile([1, 1], f32)
            nc.vector.tensor_copy(res[:], ps[:])
            nc.sync.dma_start(out=o_d[:], in_=res[:])

    nc.compile()
    return nc


def _get_nc():
    if "nc" not in _cache:
        _cache["nc"] = _build_nc()
    return _cache["nc"]


def _prep(x, labels):
    import ml_dtypes
    bf16 = np.dtype(ml_dtypes.bfloat16)
    fp8 = np.dtype(ml_dtypes.float8_e4m3fn)
    x = np.asarray(x, dtype=np.float32).reshape(-1)
    t = np.asarray(labels).reshape(-1) > 0

    xf = np.full(TOTAL_PAD, X_PAD, dtype=bf16)
    xf[:TOTAL] = x.astype(bf16)
    tf = np.zeros(TOTAL_PAD, dtype=fp8)
    tf[:TOTAL] = t.astype(fp8)
    return xf.reshape(NCORES, P, F), tf.reshape(NCORES, P, F)


def kernel(x, labels, _trace=False):
    from concourse.bass_utils import run_bass_kernel_spmd

    xs, ts = _prep(x, labels)
    nc = _get_nc()
    in_maps = [{"x": xs[c], "t": ts[c]} for c in range(NCORES)]
    r = run_bass_kernel_spmd(nc, in_maps, list(range(NCORES)), trace=_trace)
    total = sum(float(r.results[c]["o"][0, 0]) for c in range(NCORES))
    out = np.asarray(total, dtype=np.float32)
    if _trace:
        _cache["last_results"] = r
    return out
